# revision 1
# baseline (speedup 1.0000x reference)
"""NeRF-NGP MLP kernel for Trainium2 (8 NeuronCores, pure data parallel).

Network (bias-free, fp32 reference):
  sigma net: x[:, :32] -> 64 -> 64 -> (1 sigma + 15 geo)
  color net: concat(x[:, 32:48], geo) -> 64 -> 64 -> 64 -> 3
  out = [color(3), sigma(1)]   shape [N, 4]

Device strategy (per core, N_CORE = 262144 points):
  - Activations live "layout B": [channels(partitions), points(free)].
  - Every layer is matmul(psum[M,512], lhsT=W[K,M], rhs=act[K,512]).
  - The concat is algebraically fused away on the host:
      W3  = s2[:,1:] @ c0[16:,:]   (geo path, 64x64)
      W3v = c0[:16,:] placed at partition rows 32:48 (views path)
    so  h3 = relu(W3.T @ h2 + W3v.T @ x_chans)   via PSUM accumulation.
    sigma is folded into the final layer the same way:
      out4 = W6a.T @ h5 + W6b.T @ h2  with W6a=[c3|0], W6b=[0|s2[:,0]].
  - 4-way PE-array packing: 64x64 tile_position quadrants; 4 chunks of 512
    points ("u,v,w,z") advance through the layers with a rotation schedule
    that keeps every matmul's rhs in the row group its tile reads.
  - Matmul operands in fp16 (1 cyc/col on the PE vs 4 for fp32; 11-bit
    mantissa adds ~6e-4 absmax-relative error end to end). PSUM stays fp32.
  - PSUM evacuation (+relu) in full 128-partition [128, 1024] ops,
    alternating ScalarE / VectorE per layer.
  - ILV groups are software-pipelined (emission round-robin) so each
    engine's in-order stream has independent work during evac waits.
  - Input is host-pre-transposed into a blocked layout so DMA bursts are
    contiguous per partition; output is returned blocked and un-blocked
    on the host.
"""

import numpy as np

import concourse.bacc as bacc
import concourse.mybir as mybir
import concourse.tile as tile
from concourse.bass_utils import run_bass_kernel_spmd

F32 = mybir.dt.float32
RELU = mybir.ActivationFunctionType.Relu

N_PTS = 2097152
N_CORES = 8
N_CORE = N_PTS // N_CORES      # 262144
T = 512                        # points per chunk = one PSUM bank of fp32
CHUNKS_PER_GROUP = 4
PTS_PER_GROUP = T * CHUNKS_PER_GROUP   # 2048
G = N_CORE // PTS_PER_GROUP            # 128 groups per core

# matmul operand dtype: float16 (1 cyc/col) or float32 (exact, 4 cyc/col)
MM_DT = mybir.dt.float16
ILV = 4        # groups software-pipelined together

# tile name -> (rhs row-group base, psum col-position base)
TILES = {"T0": (0, 0), "T2": (0, 64), "T8": (64, 0), "T10": (64, 64)}
# tile name -> which 512-wide window of the psum/h tile the result lands in
PWIN = {"T0": 0, "T2": 0, "T8": 1, "T10": 1}

# per-layer chunk->tile assignment (chunks 0..3 = u,v,w,z).  Derived so that
#  - a chunk's rhs row group always matches its tile's row group,
#  - positions at L3 equal the xt positions (views + h2 reuse),
#  - row groups at L6 equal those at L3 (h5 and h2 read together).
SCHED = [
    {0: "T0", 2: "T2", 1: "T8", 3: "T10"},   # L1
    {0: "T0", 1: "T2", 2: "T8", 3: "T10"},   # L2
    {0: "T0", 2: "T2", 1: "T8", 3: "T10"},   # L3 (2 matmuls per chunk)
    {1: "T0", 0: "T2", 2: "T8", 3: "T10"},   # L4
    {2: "T0", 1: "T2", 0: "T8", 3: "T10"},   # L5
    {0: "T0", 2: "T2", 1: "T8", 3: "T10"},   # L6 (2 matmuls per chunk)
]

# weight free-dim offsets inside the [128, 512] weight tile
WCOL = {"W1": 0, "W3v": 64, "W2": 128, "W3": 192, "W4": 256, "W5": 320,
        "W6a": 384, "W6b": 448}

_PROG = {}


def _np_mm_dt():
    return np.float16 if MM_DT == mybir.dt.float16 else np.float32


def _build_program(g_count, passes=1, hbufs=None, xbufs=None, obufs=3,
                   pbufs=4, ilv=None):
    if ilv is None:
        ilv = ILV
    if hbufs is None:
        hbufs = ilv + 1
    if xbufs is None:
        xbufs = ilv + 1
    mdt = MM_DT
    nc = bacc.Bacc()
    xp = nc.dram_tensor("xp", [g_count, 2, 48, 2, T], mdt, kind="ExternalInput")
    wt = nc.dram_tensor("wt", [128, 512], mdt, kind="ExternalInput")
    od = nc.dram_tensor("od", [g_count, 2, 4, 2, T], F32, kind="ExternalOutput")

    with tile.TileContext(nc) as tc:
        with (
            tc.tile_pool(name="wp", bufs=1) as wp,
            tc.tile_pool(name="xtp", bufs=xbufs) as xtp,
            tc.tile_pool(name="h1p", bufs=hbufs) as h1p,
            tc.tile_pool(name="h2p", bufs=hbufs) as h2p,
            tc.tile_pool(name="h3p", bufs=hbufs) as h3p,
            tc.tile_pool(name="h4p", bufs=hbufs) as h4p,
            tc.tile_pool(name="h5p", bufs=hbufs) as h5p,
            tc.tile_pool(name="osp", bufs=obufs) as osp,
            tc.tile_pool(name="pp", bufs=pbufs, space="PSUM") as pp,
        ):
            hpools = [h1p, h2p, h3p, h4p, h5p]
            w = wp.tile([128, 512], mdt)
            nc.sync.dma_start(out=w, in_=wt[:, :])

            def wsl(name, rg, k):
                c = WCOL[name]
                return w[rg: rg + k, c: c + 64]

            def emit_step(st, L, g):
                ps = pp.tile([128, 2, T], F32)
                xt, pos, hs = st["xt"], st["pos"], st["hs"]
                prev = hs[L - 1] if L > 0 else None
                for c, tname in SCHED[L].items():
                    rg, colpos = TILES[tname]
                    pwin = PWIN[tname]
                    crg, cwin = pos[c]
                    assert crg == rg, (g, L, c, tname, pos)
                    out_ap = ps[colpos: colpos + 64, pwin]
                    tp = (rg, colpos)
                    if L == 0:
                        nc.tensor.matmul(
                            out=out_ap, lhsT=wsl("W1", rg, 48),
                            rhs=xt[crg: crg + 48, cwin],
                            start=True, stop=True, tile_position=tp)
                    elif L == 2:
                        nc.tensor.matmul(
                            out=out_ap, lhsT=wsl("W3", rg, 64),
                            rhs=hs[1][crg: crg + 64, cwin],
                            start=True, stop=False, tile_position=tp)
                        xrg, xwin = st["xt_pos"][c]
                        assert (xrg, xwin) == (crg, cwin)
                        nc.tensor.matmul(
                            out=out_ap, lhsT=wsl("W3v", rg, 48),
                            rhs=xt[xrg: xrg + 48, xwin],
                            start=False, stop=True, tile_position=tp)
                    elif L == 5:
                        nc.tensor.matmul(
                            out=out_ap, lhsT=wsl("W6a", rg, 64),
                            rhs=hs[4][crg: crg + 64, cwin],
                            start=True, stop=False, tile_position=tp)
                        h2rg, h2win = st["h2_pos"][c]
                        assert h2rg == crg
                        nc.tensor.matmul(
                            out=out_ap, lhsT=wsl("W6b", rg, 64),
                            rhs=hs[1][h2rg: h2rg + 64, h2win],
                            start=False, stop=True, tile_position=tp)
                    else:
                        wname = {1: "W2", 3: "W4", 4: "W5"}[L]
                        nc.tensor.matmul(
                            out=out_ap, lhsT=wsl(wname, rg, 64),
                            rhs=prev[crg: crg + 64, cwin],
                            start=True, stop=True, tile_position=tp)
                    pos[c] = (colpos, pwin)

                if L < 5:
                    h = hpools[L].tile([128, 2, T], mdt)
                    on_act = L in (0, 2, 4) or (L == 3 and g % 3 == 0)
                    if on_act:
                        nc.scalar.activation(h[:, :, :], ps[:, :, :], RELU)
                    else:
                        nc.vector.tensor_scalar_max(h[:, :, :], ps[:, :, :], 0.0)
                    hs.append(h)
                    if L == 1:
                        st["h2_pos"] = dict(pos)
                else:
                    osb = osp.tile([128, 2, T], F32)
                    nc.vector.tensor_copy(osb[:, :, :], ps[:, :, :])
                    nc.sync.dma_start(out=od[g, 0], in_=osb[0:4])
                    nc.sync.dma_start(out=od[g, 1], in_=osb[64:68])

            glist = [g for _ in range(passes) for g in range(g_count)]
            for gbase in range(0, len(glist), ilv):
                block = glist[gbase: gbase + ilv]
                st = {}
                for g in block:
                    xt = xtp.tile([128, 2, T], mdt)
                    nc.sync.dma_start(out=xt[0:48], in_=xp[g, 0])
                    nc.sync.dma_start(out=xt[64:112], in_=xp[g, 1])
                    st[g] = {
                        "xt": xt,
                        "pos": {0: (0, 0), 1: (64, 0), 2: (0, 1), 3: (64, 1)},
                        "hs": [],
                    }
                    st[g]["xt_pos"] = dict(st[g]["pos"])
                for L in range(6):
                    for g in block:
                        emit_step(st[g], L, g)

    nc.finalize()
    return nc


def _get_program():
    if "nc" not in _PROG:
        _PROG["nc"] = _build_program(G)
    return _PROG["nc"]


def _build_weights(s0, s1, s2, c0, c1, c2, c3):
    w = np.zeros((64, 512), np.float32)
    w[0:32, WCOL["W1"]: WCOL["W1"] + 64] = s0
    w[32:48, WCOL["W3v"]: WCOL["W3v"] + 64] = c0[:16]
    w[0:64, WCOL["W2"]: WCOL["W2"] + 64] = s1
    w[0:64, WCOL["W3"]: WCOL["W3"] + 64] = (
        s2[:, 1:].astype(np.float64) @ c0[16:].astype(np.float64)
    ).astype(np.float32)
    w[0:64, WCOL["W4"]: WCOL["W4"] + 64] = c1
    w[0:64, WCOL["W5"]: WCOL["W5"] + 64] = c2
    w[0:64, WCOL["W6a"]: WCOL["W6a"] + 3] = c3
    w[0:64, WCOL["W6b"] + 3] = s2[:, 0]
    return np.concatenate([w, w], axis=0)


def kernel(x, s0, s1, s2, c0, c1, c2, c3):
    x = np.asarray(x, dtype=np.float32)
    assert x.shape == (N_PTS, 48), x.shape
    args = [np.asarray(a, dtype=np.float32) for a in (s0, s1, s2, c0, c1, c2, c3)]
    w_host = _build_weights(*args).astype(_np_mm_dt())

    in_maps = []
    for i in range(N_CORES):
        xc = x[i * N_CORE: (i + 1) * N_CORE]
        xprep = np.ascontiguousarray(
            xc.reshape(G, 2, 2, T, 48).transpose(0, 2, 4, 1, 3)
        ).astype(_np_mm_dt())
        in_maps.append({"xp": xprep, "wt": w_host})

    nc = _get_program()
    res = run_bass_kernel_spmd(nc, in_maps, core_ids=list(range(N_CORES)))

    outs = []
    for i in range(N_CORES):
        od = res.results[i]["od"]
        outs.append(od.transpose(0, 1, 3, 4, 2).reshape(N_CORE, 4))
    return np.concatenate(outs, axis=0)



# revision 3
# speedup vs baseline: 1.4146x; 1.4146x over previous
"""NeRF-NGP MLP kernel for Trainium2 (8 NeuronCores, pure data parallel).

Per core (262144 points, superslot = 1024 points = 2 chunks, G = 256):
PE runs 4 K-packed fp16 matmul passes per chunk, PSUM-accumulation
fusing the concat and keeping every evacuation full-width:
  I1a(u): [pts(u); v(u-2)] -> [h1(u); v-part of h3(u-2)]   (K=48, M=128)
  I1b(u): accumulate W3.h2(u-2) into the h3 half           (K=64, same bank)
  I2(u):  [h1(u); h3(u-2)] -> [h2(u)(0:64); h4(u-2)(64:128)] (K=128, M=128)
  I3:     h4 -> h5                                         (K=64,  M=64)
sigma = s2[:,0].h2 and color = c3.h5 are stationary-side matmuls
(activation slices as lhsT, weight vectors as rhs, out free dim 1/3)
accumulated into a PSUM bank drained every 16 superslots.

PSUM->SBUF evacuations (relu + fp32->fp16) are the bottleneck and are
load-balanced across ScalarE and VectorE; every evac is a full-width
[128, 512] partition-preserving op writing directly into consumer rhs
tiles (D = [h2; h4] feeds I1b, sigma, and I3). h5 psums of consecutive
superslots are pair-packed into one bank at complementary partition
halves (tile_position col 0/64) so their evac is one full-width op.
Emission is stage-skewed so every PE-consumes-evac edge crosses an
iteration boundary, and per-chunk one-bank psum tiles with bufs=2 give
every psum reuse two chunk-steps of slack. All matmuls of one
accumulation group share the same tile_position row group (the device
path rejects cross-row-group accumulation).
"""

import numpy as np

import concourse.bacc as bacc
import concourse.mybir as mybir
import concourse.tile as tile
from concourse.bass_utils import run_bass_kernel_spmd

F32 = mybir.dt.float32
F16 = mybir.dt.float16
RELU = mybir.ActivationFunctionType.Relu

N_PTS = 2097152
N_CORES = 8
N_CORE = N_PTS // N_CORES      # 262144
T = 512
SS = 1024                      # points per superslot (2 chunks)
G = N_CORE // SS               # 256 superslots
M_MEGA = G // 2                # input DMA batches (2 superslots each)
NGEN = G // 16                 # sigma/color psum generations

# evac scheduling: alternate ACT/DVE weighted by their op costs
# (ACT [*,1024] = 1038 ns, DVE = 1192 ns -> ACT share ~53.5%)
PAT = "ADADADADADADADADADADADADADAA"       # 28-cycle: A=15, D=13

_PROG = {}


def _build_program(g=None):
    GG = G if g is None else g
    nc = bacc.Bacc()
    mm_ = GG // 2
    ngen_ = max(GG // 16, 1)
    xin = nc.dram_tensor("xin", [mm_ + 1, 64, 2, 2, T], F16,
                         kind="ExternalInput")
    wt = nc.dram_tensor("wt", [128, 512], F16, kind="ExternalInput")
    pcd = nc.dram_tensor("pcd", [ngen_, 128, 512], F32, kind="ExternalOutput")

    with tile.TileContext(nc) as tc:
        with (
            tc.tile_pool(name="wp", bufs=1) as wp,
            tc.tile_pool(name="xp", bufs=5) as xp,
            tc.tile_pool(name="rp", bufs=3) as rp,
            tc.tile_pool(name="up", bufs=4) as up,
            tc.tile_pool(name="hp", bufs=3) as hp,
            tc.tile_pool(name="scp", bufs=3) as scp,
            tc.tile_pool(name="p1p", bufs=2, space="PSUM") as p1p,
            tc.tile_pool(name="p2p", bufs=2, space="PSUM") as p2p,
            tc.tile_pool(name="p3p", bufs=2, space="PSUM") as p3p,
            tc.tile_pool(name="pcp", bufs=2, space="PSUM") as pcp,
        ):
            w = wp.tile([128, 512], F16)
            nc.sync.dma_start(out=w, in_=wt[:, :])

            xtiles = {}            # mega index -> tile [128, 2, 2, T]
            def ensure_mega(m):
                if m in xtiles and m <= mm_:
                    return
                q = xp.tile([128, 2, 2, T], F16, name="xm")
                nc.sync.dma_start(out=q[0:64], in_=xin[m])
                xtiles[m] = q
            def xsl(u):
                # [128, 2, T] view of superslot u
                return xtiles[u // 2][:, u % 2]

            ensure_mega(0)

            ev_i = [0]
            def evac(ps_ap, dst_ap, width=None):
                """relu+cast psum->sbuf, round-robin ACT/DVE."""
                k = PAT[ev_i[0] % len(PAT)]
                ev_i[0] += 1
                if k == "A":
                    nc.scalar.activation(dst_ap, ps_ap, RELU)
                else:
                    nc.vector.tensor_scalar_max(dst_ap, ps_ap, 0.0)

            rt = {}; dt = {}; pc_tiles = {}; p3_hold = [None]
            h5_hold = [None]
            for k in range(GG + 5):
                # prefetch input megas for slots k..k+2
                for m in ((k + 1) // 2, (k + 2) // 2):
                    if m <= mm_:
                        ensure_mega(m)

                # --- I1(a=k) + E1: I1a (x -> h1 + v-part of h3),
                #     I1b accumulates W3.h2 into the h3 half ---
                if k <= GG + 1:
                    r = rp.tile([128, 2, T], F16)
                    rt[k] = r
                    for c in range(2):
                        p1 = p1p.tile([128, T], F32, name="p1")
                        nc.tensor.matmul(
                            out=p1[:, :], lhsT=w[0:48, 0:128],
                            rhs=xsl(k)[0:48, c], start=True, stop=(k < 2),
                            tile_position=(0, 0))
                        if k >= 2:
                            nc.tensor.matmul(
                                out=p1[:, :], lhsT=w[0:64, 128:256],
                                rhs=dt[k - 2][0:64, c], start=False,
                                stop=True, tile_position=(0, 0))
                        evac(p1[:, :], r[:, c])

                # --- I2(b=k-1) + merged E2: D(b) = [h4(b-2); h2(b)] ---
                b = k - 1
                if 0 <= b <= GG + 1:
                    dd = up.tile([128, 2, T], F16)
                    dt[b] = dd
                    for c in range(2):
                        p2 = p2p.tile([128, T], F32, name="p2")
                        nc.tensor.matmul(
                            out=p2[:, :], lhsT=w[0:128, 256:384],
                            rhs=rt[b][:, c], start=True, stop=True,
                            tile_position=(0, 0))
                        evac(p2[:, :], dd[:, c])

                # --- sigma(s=k-2) ---
                s_ = k - 2
                if 0 <= s_ <= GG - 1:
                    if s_ % 16 == 0:
                        pc_tiles[s_ // 16] = pcp.tile([128, 512], F32,
                                                      name="pcb")
                    xz = dt[s_]
                    pc = pc_tiles[s_ // 16]
                    cb = (s_ % 16) * 32
                    for i in range(8):
                        c, j = i // 4, i % 4
                        nc.tensor.matmul(
                            out=pc[:, cb + i: cb + i + 1],
                            lhsT=xz[0:64, c, j * 128:(j + 1) * 128],
                            rhs=w[0:64, 448:449],
                            start=True, stop=True, tile_position=(0, 0))

                # --- I3(e=k-4): h5 psums pair-packed at partition halves ---
                e = k - 4
                if 0 <= e <= GG - 1:
                    if e % 2 == 0:
                        p3_hold[0] = [p3p.tile([128, T], F32, name="p3")
                                      for _ in range(2)]
                        lo, tp = 0, (0, 0)
                    else:
                        lo, tp = 64, (0, 64)
                    for c in range(2):
                        nc.tensor.matmul(
                            out=p3_hold[0][c][lo:lo + 64, :],
                            lhsT=w[64:128, 384:448],
                            rhs=dt[k - 2][64:128, c], start=True, stop=True,
                            tile_position=(64, tp[1]))
                    if e % 2 == 1:
                        h5p = hp.tile([128, 2, T], F16)
                        for c in range(2):
                            evac(p3_hold[0][c][:, :], h5p[:, c])
                        h5_hold[0] = (e, h5p)

                # --- color for the pair finished last iteration ---
                if h5_hold[0] is not None and h5_hold[0][0] == k - 5:
                    eo, h5p = h5_hold[0]
                    for w2, lo2 in ((eo - 1, 0), (eo, 64)):
                        pc = pc_tiles[w2 // 16]
                        cb = (w2 % 16) * 32 + 8
                        for i in range(8):
                            c, j = i // 4, i % 4
                            nc.tensor.matmul(
                                out=pc[:, cb + 3 * i: cb + 3 * i + 3],
                                lhsT=h5p[lo2:lo2 + 64, c,
                                         j * 128:(j + 1) * 128],
                                rhs=w[lo2:lo2 + 64, 449:452],
                                start=True, stop=True,
                                tile_position=(lo2, 0))

                # --- PC bank drain ---
                if k >= 20 and (k - 20) % 16 == 0:
                    g = (k - 20) // 16
                    pcs = scp.tile([128, 512], F32, name="pcs")
                    nc.vector.tensor_copy(pcs[:, :], pc_tiles[g][:, :])
                    nc.sync.dma_start(out=pcd[g], in_=pcs[:, :])

                rt.pop(k - 2, None)
                dt.pop(k - 3, None)
                xtiles.pop(k // 2 - 3, None)
    nc.finalize()
    return nc


def _get_program():
    if "nc" not in _PROG:
        _PROG["nc"] = _build_program()
    return _PROG["nc"]


def _build_weights(s0, s1, s2, c0, c1, c2, c3):
    w = np.zeros((128, 512), np.float32)
    W3 = (s2[:, 1:].astype(np.float64) @ c0[16:].astype(np.float64)
          ).astype(np.float32)
    # L1a (cols 0:128), K-rows = [pts(0:32); views(32:48)]
    w[0:32, 0:64] = s0
    w[32:48, 64:128] = c0[:16]
    # L1b (cols 128:256), K-rows 0:64 = h2 -> W3 into h3 (out cols 64:128)
    w[0:64, 128 + 64:256] = W3
    # L2 (cols 256:384): h1 -> h2 (out 0:64), h3 -> h4 (out 64:128)
    w[0:64, 256:320] = s1
    w[64:128, 320:384] = c1
    # L3/W5 (cols 384:448), K-rows 64:128 = h4 -> h5
    w[64:128, 384:448] = c2
    # sigma vector (rows 0:64 = h2), color c3 at both halves
    w[0:64, 448] = s2[:, 0]
    w[0:64, 449:452] = c3
    w[64:128, 449:452] = c3
    return w.astype(np.float16)


def _pack_input(xc):
    """xc [N_CORE, 48] f32 -> xin [M_MEGA+1, 48, 2, 2, T] f16.
    xin[m, 0:32, s, c, t]  = pts of superslot u=2m+s
    xin[m, 32:48, s, c, t] = views of superslot u-2 (zeros for u<2)."""
    xr = xc.reshape(G, 2, T, 48)
    xin = np.zeros((M_MEGA + 1, 64, 2, 2, T), np.float16)
    pts = xr[:, :, :, 0:32].transpose(0, 3, 1, 2).astype(np.float16)  # [G,32,2,T]
    vws = xr[:, :, :, 32:48].transpose(0, 3, 1, 2).astype(np.float16)
    pts_pad = np.concatenate([pts, np.zeros((2, 32, 2, T), np.float16)])
    vws_pad = np.concatenate([np.zeros((2, 16, 2, T), np.float16), vws])
    xin[:, 0:32] = pts_pad.reshape(M_MEGA + 1, 2, 32, 2, T).transpose(0, 2, 1, 3, 4)
    xin[:, 32:48] = vws_pad[:2 * (M_MEGA + 1)].reshape(
        M_MEGA + 1, 2, 16, 2, T).transpose(0, 2, 1, 3, 4)
    return xin


def _unpack_output(pcd):
    """pcd [NGEN, 128, 512] f32 -> out [N_CORE, 4] (color0..2, sigma)."""
    q = pcd.reshape(NGEN, 128, 16, 32)
    sig = q[:, :, :, 0:8].transpose(0, 2, 3, 1).reshape(N_CORE)
    col = q[:, :, :, 8:32].reshape(NGEN, 128, 16, 8, 3)
    col = col.transpose(0, 2, 3, 1, 4).reshape(N_CORE, 3)
    return np.concatenate([col, sig[:, None]], axis=1)


def kernel(x, s0, s1, s2, c0, c1, c2, c3):
    x = np.asarray(x, dtype=np.float32)
    assert x.shape == (N_PTS, 48), x.shape
    args = [np.asarray(a, dtype=np.float32) for a in (s0, s1, s2, c0, c1, c2, c3)]
    w_host = _build_weights(*args)

    in_maps = []
    for i in range(N_CORES):
        xc = x[i * N_CORE: (i + 1) * N_CORE]
        in_maps.append({"xin": _pack_input(xc), "wt": w_host})

    nc = _get_program()
    res = run_bass_kernel_spmd(nc, in_maps, core_ids=list(range(N_CORES)))

    outs = []
    for i in range(N_CORES):
        outs.append(_unpack_output(res.results[i]["pcd"]))
    return np.concatenate(outs, axis=0).astype(np.float32)


# revision 4
# speedup vs baseline: 1.4367x; 1.0157x over previous
"""NeRF-NGP MLP kernel for Trainium2 (8 NeuronCores, pure data parallel).

Per core (262144 points, superslot = 1024 points = 2 chunks, G = 256):
PE runs 4 K-packed fp16 matmul passes per chunk, PSUM-accumulation
fusing the concat and keeping every evacuation full-width:
  I1a(u): [pts(u); v(u-2)] -> [h1(u); v-part of h3(u-2)]   (K=48, M=128)
  I1b(u): accumulate W3.h2(u-2) into the h3 half           (K=64, same bank)
  I2(u):  [h1(u); h3(u-2)] -> [h2(u)(0:64); h4(u-2)(64:128)] (K=128, M=128)
  I3:     h4 -> h5                                         (K=64,  M=64)
sigma = s2[:,0].h2 and color = c3.h5 are stationary-side matmuls
(activation slices as lhsT, weight vectors as rhs, out free dim 1/3)
accumulated into a PSUM bank drained every 16 superslots.

PSUM->SBUF evacuations (relu + fp32->fp16) are the bottleneck and are
load-balanced across ScalarE and VectorE; every evac is a full-width
[128, 512] partition-preserving op writing directly into consumer rhs
tiles (D = [h2; h4] feeds I1b, sigma, and I3). h5 psums of consecutive
superslots are pair-packed into one bank at complementary partition
halves (tile_position col 0/64) so their evac is one full-width op.
Emission is stage-skewed so every PE-consumes-evac edge crosses an
iteration boundary, and per-chunk one-bank psum tiles with bufs=2 give
every psum reuse two chunk-steps of slack. All matmuls of one
accumulation group share the same tile_position row group (the device
path rejects cross-row-group accumulation).
"""

import numpy as np

import concourse.bacc as bacc
import concourse.mybir as mybir
import concourse.tile as tile
from concourse.bass_utils import run_bass_kernel_spmd

F32 = mybir.dt.float32
F16 = mybir.dt.float16
RELU = mybir.ActivationFunctionType.Relu

N_PTS = 2097152
N_CORES = 8
N_CORE = N_PTS // N_CORES      # 262144
T = 512
SS = 1024                      # points per superslot (2 chunks)
G = N_CORE // SS               # 256 superslots
M_MEGA = G // 2                # input DMA batches (2 superslots each)
NGEN = G // 16                 # sigma/color psum generations

# evac scheduling: alternate ACT/DVE weighted by their op costs
# (ACT [*,1024] = 1038 ns, DVE = 1192 ns -> ACT share ~53.5%)
PAT = "ADADADADADADADADADADADADADA"        # 27-cycle: A=14, D=13

_PROG = {}


def _build_program(g=None):
    GG = G if g is None else g
    nc = bacc.Bacc()
    mm_ = GG // 2
    ngen_ = max(GG // 16, 1)
    xin = nc.dram_tensor("xin", [mm_ + 1, 64, 2, 2, T], F16,
                         kind="ExternalInput")
    wt = nc.dram_tensor("wt", [128, 512], F16, kind="ExternalInput")
    pcd = nc.dram_tensor("pcd", [ngen_, 128, 512], F32, kind="ExternalOutput")

    with tile.TileContext(nc) as tc:
        with (
            tc.tile_pool(name="wp", bufs=1) as wp,
            tc.tile_pool(name="xp", bufs=5) as xp,
            tc.tile_pool(name="rp", bufs=3) as rp,
            tc.tile_pool(name="up", bufs=4) as up,
            tc.tile_pool(name="hp", bufs=3) as hp,
            tc.tile_pool(name="scp", bufs=3) as scp,
            tc.tile_pool(name="p1p", bufs=2, space="PSUM") as p1p,
            tc.tile_pool(name="p2p", bufs=2, space="PSUM") as p2p,
            tc.tile_pool(name="p3p", bufs=2, space="PSUM") as p3p,
            tc.tile_pool(name="pcp", bufs=2, space="PSUM") as pcp,
        ):
            w = wp.tile([128, 512], F16)
            nc.sync.dma_start(out=w, in_=wt[:, :])

            xtiles = {}            # mega index -> tile [128, 2, 2, T]
            def ensure_mega(m):
                if m in xtiles and m <= mm_:
                    return
                q = xp.tile([128, 2, 2, T], F16, name="xm")
                nc.sync.dma_start(out=q[0:64], in_=xin[m])
                xtiles[m] = q
            def xsl(u):
                # [128, 2, T] view of superslot u
                return xtiles[u // 2][:, u % 2]

            ensure_mega(0)

            ev_i = [0]
            def evac(ps_ap, dst_ap, width=None):
                """relu+cast psum->sbuf, round-robin ACT/DVE."""
                k = PAT[ev_i[0] % len(PAT)]
                ev_i[0] += 1
                if k == "A":
                    nc.scalar.activation(dst_ap, ps_ap, RELU)
                else:
                    nc.vector.tensor_scalar_max(dst_ap, ps_ap, 0.0)

            rt = {}; dt = {}; pc_tiles = {}; p3_hold = [None]
            h5_hold = [None]
            for k in range(GG + 5):
                # prefetch input megas for slots k..k+2
                for m in ((k + 1) // 2, (k + 2) // 2):
                    if m <= mm_:
                        ensure_mega(m)

                # --- I1(a=k) + E1: I1a (x -> h1 + v-part of h3),
                #     I1b accumulates W3.h2 into the h3 half ---
                if k <= GG + 1:
                    r = rp.tile([128, 2, T], F16)
                    rt[k] = r
                    for c in range(2):
                        p1 = p1p.tile([128, T], F32, name="p1")
                        nc.tensor.matmul(
                            out=p1[:, :], lhsT=w[0:48, 0:128],
                            rhs=xsl(k)[0:48, c], start=True, stop=(k < 2),
                            tile_position=(0, 0))
                        if k >= 2:
                            nc.tensor.matmul(
                                out=p1[:, :], lhsT=w[0:64, 128:256],
                                rhs=dt[k - 2][0:64, c], start=False,
                                stop=True, tile_position=(0, 0))
                        evac(p1[:, :], r[:, c])

                # --- I2(b=k-1) + merged E2: D(b) = [h4(b-2); h2(b)] ---
                b = k - 1
                if 0 <= b <= GG + 1:
                    dd = up.tile([128, 2, T], F16)
                    dt[b] = dd
                    for c in range(2):
                        p2 = p2p.tile([128, T], F32, name="p2")
                        nc.tensor.matmul(
                            out=p2[:, :], lhsT=w[0:128, 256:384],
                            rhs=rt[b][:, c], start=True, stop=True,
                            tile_position=(0, 0))
                        evac(p2[:, :], dd[:, c])

                # --- sigma(s=k-2) ---
                s_ = k - 2
                if 0 <= s_ <= GG - 1:
                    if s_ % 16 == 0:
                        pc_tiles[s_ // 16] = pcp.tile([128, 512], F32,
                                                      name="pcb")
                    xz = dt[s_]
                    pc = pc_tiles[s_ // 16]
                    cb = (s_ % 16) * 32
                    for i in range(8):
                        c, j = i // 4, i % 4
                        nc.tensor.matmul(
                            out=pc[:, cb + i: cb + i + 1],
                            lhsT=xz[0:64, c, j * 128:(j + 1) * 128],
                            rhs=w[0:64, 448:449],
                            start=True, stop=True, tile_position=(0, 0))

                # --- I3(e=k-4): h5 psums pair-packed at partition halves ---
                e = k - 4
                if 0 <= e <= GG - 1:
                    if e % 2 == 0:
                        p3_hold[0] = [p3p.tile([128, T], F32, name="p3")
                                      for _ in range(2)]
                        lo, tp = 0, (0, 0)
                    else:
                        lo, tp = 64, (0, 64)
                    for c in range(2):
                        nc.tensor.matmul(
                            out=p3_hold[0][c][lo:lo + 64, :],
                            lhsT=w[64:128, 384:448],
                            rhs=dt[k - 2][64:128, c], start=True, stop=True,
                            tile_position=(64, tp[1]))
                    if e % 2 == 1:
                        h5p = hp.tile([128, 2, T], F16)
                        for c in range(2):
                            evac(p3_hold[0][c][:, :], h5p[:, c])
                        h5_hold[0] = (e, h5p)

                # --- color for the pair finished last iteration ---
                if h5_hold[0] is not None and h5_hold[0][0] == k - 5:
                    eo, h5p = h5_hold[0]
                    for w2, lo2 in ((eo - 1, 0), (eo, 64)):
                        pc = pc_tiles[w2 // 16]
                        cb = (w2 % 16) * 32 + 8
                        for i in range(8):
                            c, j = i // 4, i % 4
                            nc.tensor.matmul(
                                out=pc[:, cb + 3 * i: cb + 3 * i + 3],
                                lhsT=h5p[lo2:lo2 + 64, c,
                                         j * 128:(j + 1) * 128],
                                rhs=w[lo2:lo2 + 64, 449:452],
                                start=True, stop=True,
                                tile_position=(lo2, 0))

                # --- PC bank drain ---
                if k >= 20 and (k - 20) % 16 == 0:
                    g = (k - 20) // 16
                    pcs = scp.tile([128, 512], F32, name="pcs")
                    nc.vector.tensor_copy(pcs[:, :], pc_tiles[g][:, :])
                    nc.sync.dma_start(out=pcd[g], in_=pcs[:, :])

                rt.pop(k - 2, None)
                dt.pop(k - 3, None)
                xtiles.pop(k // 2 - 3, None)
    nc.finalize()
    return nc


def _get_program():
    if "nc" not in _PROG:
        _PROG["nc"] = _build_program()
    return _PROG["nc"]


def _build_weights(s0, s1, s2, c0, c1, c2, c3):
    w = np.zeros((128, 512), np.float32)
    W3 = (s2[:, 1:].astype(np.float64) @ c0[16:].astype(np.float64)
          ).astype(np.float32)
    # L1a (cols 0:128), K-rows = [pts(0:32); views(32:48)]
    w[0:32, 0:64] = s0
    w[32:48, 64:128] = c0[:16]
    # L1b (cols 128:256), K-rows 0:64 = h2 -> W3 into h3 (out cols 64:128)
    w[0:64, 128 + 64:256] = W3
    # L2 (cols 256:384): h1 -> h2 (out 0:64), h3 -> h4 (out 64:128)
    w[0:64, 256:320] = s1
    w[64:128, 320:384] = c1
    # L3/W5 (cols 384:448), K-rows 64:128 = h4 -> h5
    w[64:128, 384:448] = c2
    # sigma vector (rows 0:64 = h2), color c3 at both halves
    w[0:64, 448] = s2[:, 0]
    w[0:64, 449:452] = c3
    w[64:128, 449:452] = c3
    return w.astype(np.float16)


def _pack_input(xc):
    """xc [N_CORE, 48] f32 -> xin [M_MEGA+1, 48, 2, 2, T] f16.
    xin[m, 0:32, s, c, t]  = pts of superslot u=2m+s
    xin[m, 32:48, s, c, t] = views of superslot u-2 (zeros for u<2)."""
    xr = xc.reshape(G, 2, T, 48)
    xin = np.zeros((M_MEGA + 1, 64, 2, 2, T), np.float16)
    pts = xr[:, :, :, 0:32].transpose(0, 3, 1, 2).astype(np.float16)  # [G,32,2,T]
    vws = xr[:, :, :, 32:48].transpose(0, 3, 1, 2).astype(np.float16)
    pts_pad = np.concatenate([pts, np.zeros((2, 32, 2, T), np.float16)])
    vws_pad = np.concatenate([np.zeros((2, 16, 2, T), np.float16), vws])
    xin[:, 0:32] = pts_pad.reshape(M_MEGA + 1, 2, 32, 2, T).transpose(0, 2, 1, 3, 4)
    xin[:, 32:48] = vws_pad[:2 * (M_MEGA + 1)].reshape(
        M_MEGA + 1, 2, 16, 2, T).transpose(0, 2, 1, 3, 4)
    return xin


def _unpack_output(pcd):
    """pcd [NGEN, 128, 512] f32 -> out [N_CORE, 4] (color0..2, sigma)."""
    q = pcd.reshape(NGEN, 128, 16, 32)
    sig = q[:, :, :, 0:8].transpose(0, 2, 3, 1).reshape(N_CORE)
    col = q[:, :, :, 8:32].reshape(NGEN, 128, 16, 8, 3)
    col = col.transpose(0, 2, 3, 1, 4).reshape(N_CORE, 3)
    return np.concatenate([col, sig[:, None]], axis=1)


def kernel(x, s0, s1, s2, c0, c1, c2, c3):
    x = np.asarray(x, dtype=np.float32)
    assert x.shape == (N_PTS, 48), x.shape
    args = [np.asarray(a, dtype=np.float32) for a in (s0, s1, s2, c0, c1, c2, c3)]
    w_host = _build_weights(*args)

    in_maps = []
    for i in range(N_CORES):
        xc = x[i * N_CORE: (i + 1) * N_CORE]
        in_maps.append({"xin": _pack_input(xc), "wt": w_host})

    nc = _get_program()
    res = run_bass_kernel_spmd(nc, in_maps, core_ids=list(range(N_CORES)))

    outs = []
    for i in range(N_CORES):
        outs.append(_unpack_output(res.results[i]["pcd"]))
    return np.concatenate(outs, axis=0).astype(np.float32)


# revision 5
# speedup vs baseline: 1.4743x; 1.0261x over previous
"""NeRF-NGP MLP kernel for Trainium2 (8 NeuronCores, pure data parallel).

Per core (262144 points, superslot = 1024 points = 2 chunks, G = 256):
PE runs 4 K-packed fp16 matmul passes per chunk, PSUM-accumulation
fusing the concat and keeping every evacuation full-width:
  I1a(u): [pts(u); v(u-2)] -> [h1(u); v-part of h3(u-2)]   (K=48, M=128)
  I1b(u): accumulate W3.h2(u-2) into the h3 half           (K=64, same bank)
  I2(u):  [h1(u); h3(u-2)] -> [h2(u)(0:64); h4(u-2)(64:128)] (K=128, M=128)
  I3:     h4 -> h5                                         (K=64,  M=64)
sigma = s2[:,0].h2 and color = c3.h5 are stationary-side matmuls
(activation slices as lhsT, weight vectors as rhs, out free dim 1/3)
accumulated into a PSUM bank drained every 16 superslots.

PSUM->SBUF evacuations (relu + fp32->fp16) are the bottleneck and are
load-balanced across ScalarE and VectorE; every evac is a full-width
[128, 512] partition-preserving op writing directly into consumer rhs
tiles (D = [h2; h4] feeds I1b, sigma, and I3). h5 psums of consecutive
superslots are pair-packed into one bank at complementary partition
halves (tile_position col 0/64) so their evac is one full-width op.
Emission is stage-skewed so every PE-consumes-evac edge crosses an
iteration boundary, and per-chunk one-bank psum tiles with bufs=2 give
every psum reuse two chunk-steps of slack. All matmuls of one
accumulation group share the same tile_position row group (the device
path rejects cross-row-group accumulation).
"""

import numpy as np

import concourse.bacc as bacc
import concourse.mybir as mybir
import concourse.tile as tile
from concourse.bass_utils import run_bass_kernel_spmd

F32 = mybir.dt.float32
F16 = mybir.dt.float16
RELU = mybir.ActivationFunctionType.Relu

N_PTS = 2097152
N_CORES = 8
N_CORE = N_PTS // N_CORES      # 262144
T = 512
SS = 1024                      # points per superslot (2 chunks)
G = N_CORE // SS               # 256 superslots
M_MEGA = G // 2                # input DMA batches (2 superslots each)
NGEN = G // 16                 # sigma/color psum generations

# evac scheduling: alternate ACT/DVE weighted by their op costs
# (ACT [*,1024] = 1038 ns, DVE = 1192 ns -> ACT share ~53.5%)
PAT = "ADADADADADADADADADADADADADA"        # 27-cycle: A=14, D=13

_PROG = {}


def _build_program(g=None):
    GG = G if g is None else g
    nc = bacc.Bacc()
    mm_ = GG // 2
    ngen_ = max(GG // 16, 1)
    xin = nc.dram_tensor("xin", [mm_ + 1, 64, 2, 2, T], F16,
                         kind="ExternalInput")
    wt = nc.dram_tensor("wt", [128, 512], F16, kind="ExternalInput")
    pcd = nc.dram_tensor("pcd", [ngen_, 128, 512], F32, kind="ExternalOutput")

    with tile.TileContext(nc) as tc:
        with (
            tc.tile_pool(name="wp", bufs=1) as wp,
            tc.tile_pool(name="xp", bufs=5) as xp,
            tc.tile_pool(name="rp", bufs=3) as rp,
            tc.tile_pool(name="up", bufs=5) as up,
            tc.tile_pool(name="hp", bufs=3) as hp,
            tc.tile_pool(name="scp", bufs=3) as scp,
            tc.tile_pool(name="p1p", bufs=2, space="PSUM") as p1p,
            tc.tile_pool(name="p2p", bufs=3, space="PSUM") as p2p,
            tc.tile_pool(name="p3p", bufs=2, space="PSUM") as p3p,
            tc.tile_pool(name="pcp", bufs=1, space="PSUM") as pcp,
        ):
            w = wp.tile([128, 512], F16)
            nc.sync.dma_start(out=w, in_=wt[:, :])

            xtiles = {}            # mega index -> tile [128, 2, 2, T]
            def ensure_mega(m):
                if m in xtiles and m <= mm_:
                    return
                q = xp.tile([128, 2, 2, T], F16, name="xm")
                nc.sync.dma_start(out=q[0:64], in_=xin[m])
                xtiles[m] = q
            def xsl(u):
                # [128, 2, T] view of superslot u
                return xtiles[u // 2][:, u % 2]

            ensure_mega(0)

            ev_i = [0]
            def evac(ps_ap, dst_ap, width=None):
                """relu+cast psum->sbuf, round-robin ACT/DVE."""
                k = PAT[ev_i[0] % len(PAT)]
                ev_i[0] += 1
                if k == "A":
                    nc.scalar.activation(dst_ap, ps_ap, RELU)
                else:
                    nc.vector.tensor_scalar_max(dst_ap, ps_ap, 0.0)

            rt = {}; dt = {}; pc_tiles = {}; p3_hold = [None]
            h5_hold = [None]
            for k in range(GG + 5):
                # prefetch input megas for slots k..k+2
                for m in ((k + 1) // 2, (k + 2) // 2):
                    if m <= mm_:
                        ensure_mega(m)

                # --- I1(a=k) + E1: I1a (x -> h1 + v-part of h3),
                #     I1b accumulates W3.h2 into the h3 half ---
                if k <= GG + 1:
                    r = rp.tile([128, 2, T], F16)
                    rt[k] = r
                    for c in range(2):
                        p1 = p1p.tile([128, T], F32, name="p1")
                        nc.tensor.matmul(
                            out=p1[:, :], lhsT=w[0:48, 0:128],
                            rhs=xsl(k)[0:48, c], start=True, stop=(k < 2),
                            tile_position=(0, 0))
                        if k >= 2:
                            nc.tensor.matmul(
                                out=p1[:, :], lhsT=w[0:64, 128:256],
                                rhs=dt[k - 2][0:64, c], start=False,
                                stop=True, tile_position=(0, 0))
                        evac(p1[:, :], r[:, c])

                # --- I2(b=k-1) + merged E2: D(b) = [h4(b-2); h2(b)] ---
                b = k - 1
                if 0 <= b <= GG + 1:
                    dd = up.tile([128, 2, T], F16)
                    dt[b] = dd
                    for c in range(2):
                        p2 = p2p.tile([128, T], F32, name="p2")
                        nc.tensor.matmul(
                            out=p2[:, :], lhsT=w[0:128, 256:384],
                            rhs=rt[b][:, c], start=True, stop=True,
                            tile_position=(0, 0))
                        evac(p2[:, :], dd[:, c])

                # --- sigma(s=k-2) ---
                s_ = k - 2
                if 0 <= s_ <= GG - 1:
                    if s_ % 16 == 0:
                        pc_tiles[s_ // 16] = pcp.tile([128, 512], F32,
                                                      name="pcb")
                    xz = dt[s_]
                    pc = pc_tiles[s_ // 16]
                    cb = (s_ % 16) * 32
                    for i in range(8):
                        c, j = i // 4, i % 4
                        nc.tensor.matmul(
                            out=pc[:, cb + i: cb + i + 1],
                            lhsT=xz[0:64, c, j * 128:(j + 1) * 128],
                            rhs=w[0:64, 448:449],
                            start=True, stop=True, tile_position=(0, 0))

                # --- I3(e=k-4): h5 psums pair-packed at partition halves ---
                e = k - 4
                if 0 <= e <= GG - 1:
                    if e % 2 == 0:
                        p3_hold[0] = [p3p.tile([128, T], F32, name="p3")
                                      for _ in range(2)]
                        lo, tp = 0, (0, 0)
                    else:
                        lo, tp = 64, (0, 64)
                    for c in range(2):
                        nc.tensor.matmul(
                            out=p3_hold[0][c][lo:lo + 64, :],
                            lhsT=w[64:128, 384:448],
                            rhs=dt[k - 2][64:128, c], start=True, stop=True,
                            tile_position=(64, tp[1]))
                    if e % 2 == 1:
                        h5p = hp.tile([128, 2, T], F16)
                        for c in range(2):
                            evac(p3_hold[0][c][:, :], h5p[:, c])
                        h5_hold[0] = (e, h5p)

                # --- color for the pair finished last iteration ---
                if h5_hold[0] is not None and h5_hold[0][0] == k - 5:
                    eo, h5p = h5_hold[0]
                    for w2, lo2 in ((eo - 1, 0), (eo, 64)):
                        pc = pc_tiles[w2 // 16]
                        cb = (w2 % 16) * 32 + 8
                        for i in range(8):
                            c, j = i // 4, i % 4
                            nc.tensor.matmul(
                                out=pc[:, cb + 3 * i: cb + 3 * i + 3],
                                lhsT=h5p[lo2:lo2 + 64, c,
                                         j * 128:(j + 1) * 128],
                                rhs=w[lo2:lo2 + 64, 449:452],
                                start=True, stop=True,
                                tile_position=(lo2, 0))

                # --- PC bank drain ---
                if k >= 20 and (k - 20) % 16 == 0:
                    g = (k - 20) // 16
                    pcs = scp.tile([128, 512], F32, name="pcs")
                    nc.vector.tensor_copy(pcs[:, :], pc_tiles[g][:, :])
                    nc.sync.dma_start(out=pcd[g], in_=pcs[:, :])

                rt.pop(k - 2, None)
                dt.pop(k - 3, None)
                xtiles.pop(k // 2 - 3, None)
    nc.finalize()
    return nc


def _get_program():
    if "nc" not in _PROG:
        _PROG["nc"] = _build_program()
    return _PROG["nc"]


def _build_weights(s0, s1, s2, c0, c1, c2, c3):
    w = np.zeros((128, 512), np.float32)
    W3 = (s2[:, 1:].astype(np.float64) @ c0[16:].astype(np.float64)
          ).astype(np.float32)
    # L1a (cols 0:128), K-rows = [pts(0:32); views(32:48)]
    w[0:32, 0:64] = s0
    w[32:48, 64:128] = c0[:16]
    # L1b (cols 128:256), K-rows 0:64 = h2 -> W3 into h3 (out cols 64:128)
    w[0:64, 128 + 64:256] = W3
    # L2 (cols 256:384): h1 -> h2 (out 0:64), h3 -> h4 (out 64:128)
    w[0:64, 256:320] = s1
    w[64:128, 320:384] = c1
    # L3/W5 (cols 384:448), K-rows 64:128 = h4 -> h5
    w[64:128, 384:448] = c2
    # sigma vector (rows 0:64 = h2), color c3 at both halves
    w[0:64, 448] = s2[:, 0]
    w[0:64, 449:452] = c3
    w[64:128, 449:452] = c3
    return w.astype(np.float16)


def _pack_input(xc):
    """xc [N_CORE, 48] f32 -> xin [M_MEGA+1, 48, 2, 2, T] f16.
    xin[m, 0:32, s, c, t]  = pts of superslot u=2m+s
    xin[m, 32:48, s, c, t] = views of superslot u-2 (zeros for u<2)."""
    xr = xc.reshape(G, 2, T, 48)
    xin = np.zeros((M_MEGA + 1, 64, 2, 2, T), np.float16)
    pts = xr[:, :, :, 0:32].transpose(0, 3, 1, 2).astype(np.float16)  # [G,32,2,T]
    vws = xr[:, :, :, 32:48].transpose(0, 3, 1, 2).astype(np.float16)
    pts_pad = np.concatenate([pts, np.zeros((2, 32, 2, T), np.float16)])
    vws_pad = np.concatenate([np.zeros((2, 16, 2, T), np.float16), vws])
    xin[:, 0:32] = pts_pad.reshape(M_MEGA + 1, 2, 32, 2, T).transpose(0, 2, 1, 3, 4)
    xin[:, 32:48] = vws_pad[:2 * (M_MEGA + 1)].reshape(
        M_MEGA + 1, 2, 16, 2, T).transpose(0, 2, 1, 3, 4)
    return xin


def _unpack_output(pcd):
    """pcd [NGEN, 128, 512] f32 -> out [N_CORE, 4] (color0..2, sigma)."""
    q = pcd.reshape(NGEN, 128, 16, 32)
    sig = q[:, :, :, 0:8].transpose(0, 2, 3, 1).reshape(N_CORE)
    col = q[:, :, :, 8:32].reshape(NGEN, 128, 16, 8, 3)
    col = col.transpose(0, 2, 3, 1, 4).reshape(N_CORE, 3)
    return np.concatenate([col, sig[:, None]], axis=1)


def kernel(x, s0, s1, s2, c0, c1, c2, c3):
    x = np.asarray(x, dtype=np.float32)
    assert x.shape == (N_PTS, 48), x.shape
    args = [np.asarray(a, dtype=np.float32) for a in (s0, s1, s2, c0, c1, c2, c3)]
    w_host = _build_weights(*args)

    in_maps = []
    for i in range(N_CORES):
        xc = x[i * N_CORE: (i + 1) * N_CORE]
        in_maps.append({"xin": _pack_input(xc), "wt": w_host})

    nc = _get_program()
    res = run_bass_kernel_spmd(nc, in_maps, core_ids=list(range(N_CORES)))

    outs = []
    for i in range(N_CORES):
        outs.append(_unpack_output(res.results[i]["pcd"]))
    return np.concatenate(outs, axis=0).astype(np.float32)


# revision 6
# speedup vs baseline: 1.4818x; 1.0051x over previous
"""NeRF-NGP MLP kernel for Trainium2 (8 NeuronCores, pure data parallel).

Per core (262144 points, superslot = 1024 points = 2 chunks, G = 256):
PE runs 4 K-packed fp16 matmul passes per chunk, PSUM-accumulation
fusing the concat and keeping every evacuation full-width:
  I1a(u): [pts(u); v(u-2)] -> [h1(u); v-part of h3(u-2)]   (K=48, M=128)
  I1b(u): accumulate W3.h2(u-2) into the h3 half           (K=64, same bank)
  I2(u):  [h1(u); h3(u-2)] -> [h2(u)(0:64); h4(u-2)(64:128)] (K=128, M=128)
  I3:     h4 -> h5                                         (K=64,  M=64)
sigma = s2[:,0].h2 and color = c3.h5 are stationary-side matmuls
(activation slices as lhsT, weight vectors as rhs, out free dim 1/3)
accumulated into a PSUM bank drained every 16 superslots.

PSUM->SBUF evacuations (relu + fp32->fp16) are the bottleneck and are
load-balanced across ScalarE and VectorE; every evac is a full-width
[128, 512] partition-preserving op writing directly into consumer rhs
tiles (D = [h2; h4] feeds I1b, sigma, and I3). h5 psums of consecutive
superslots are pair-packed into one bank at complementary partition
halves (tile_position col 0/64) so their evac is one full-width op.
Emission is stage-skewed so every PE-consumes-evac edge crosses an
iteration boundary, and per-chunk one-bank psum tiles with bufs=2 give
every psum reuse two chunk-steps of slack. All matmuls of one
accumulation group share the same tile_position row group (the device
path rejects cross-row-group accumulation).
"""

import numpy as np

import concourse.bacc as bacc
import concourse.mybir as mybir
import concourse.tile as tile
from concourse.bass_utils import run_bass_kernel_spmd

F32 = mybir.dt.float32
F16 = mybir.dt.float16
RELU = mybir.ActivationFunctionType.Relu

N_PTS = 2097152
N_CORES = 8
N_CORE = N_PTS // N_CORES      # 262144
T = 512
SS = 1024                      # points per superslot (2 chunks)
G = N_CORE // SS               # 256 superslots
M_MEGA = G // 2                # input DMA batches (2 superslots each)
NGEN = G // 16                 # sigma/color psum generations

# evac scheduling: alternate ACT/DVE weighted by their op costs
# (ACT [*,1024] = 1038 ns, DVE = 1192 ns -> ACT share ~53.5%)
PAT = "ADADADADADADADADADADADADAD"         # even 13:13 split

_PROG = {}


def _build_program(g=None):
    GG = G if g is None else g
    nc = bacc.Bacc()
    mm_ = GG // 2
    ngen_ = max(GG // 16, 1)
    xin = nc.dram_tensor("xin", [mm_ + 1, 64, 2, 2, T], F16,
                         kind="ExternalInput")
    wt = nc.dram_tensor("wt", [128, 512], F16, kind="ExternalInput")
    pcd = nc.dram_tensor("pcd", [ngen_, 128, 512], F32, kind="ExternalOutput")

    with tile.TileContext(nc) as tc:
        with (
            tc.tile_pool(name="wp", bufs=1) as wp,
            tc.tile_pool(name="xp", bufs=5) as xp,
            tc.tile_pool(name="rp", bufs=3) as rp,
            tc.tile_pool(name="up", bufs=5) as up,
            tc.tile_pool(name="hp", bufs=3) as hp,
            tc.tile_pool(name="scp", bufs=3) as scp,
            tc.tile_pool(name="p1p", bufs=2, space="PSUM") as p1p,
            tc.tile_pool(name="p2p", bufs=3, space="PSUM") as p2p,
            tc.tile_pool(name="p3p", bufs=2, space="PSUM") as p3p,
            tc.tile_pool(name="pcp", bufs=1, space="PSUM") as pcp,
        ):
            w = wp.tile([128, 512], F16)
            nc.sync.dma_start(out=w, in_=wt[:, :])

            xtiles = {}            # mega index -> tile [128, 2, 2, T]
            def ensure_mega(m):
                if m in xtiles and m <= mm_:
                    return
                q = xp.tile([128, 2, 2, T], F16, name="xm")
                nc.sync.dma_start(out=q[0:64], in_=xin[m])
                xtiles[m] = q
            def xsl(u):
                # [128, 2, T] view of superslot u
                return xtiles[u // 2][:, u % 2]

            ensure_mega(0)

            ev_i = [0]
            def evac(ps_ap, dst_ap, width=None):
                """relu+cast psum->sbuf, round-robin ACT/DVE."""
                k = PAT[ev_i[0] % len(PAT)]
                ev_i[0] += 1
                if k == "A":
                    nc.scalar.activation(dst_ap, ps_ap, RELU)
                else:
                    nc.vector.tensor_scalar_max(dst_ap, ps_ap, 0.0)

            rt = {}; dt = {}; pc_tiles = {}; p3_hold = [None]
            h5_hold = [None]
            for k in range(GG + 5):
                # prefetch input megas for slots k..k+2
                for m in ((k + 1) // 2, (k + 2) // 2):
                    if m <= mm_:
                        ensure_mega(m)

                # --- I1(a=k) + E1: I1a (x -> h1 + v-part of h3),
                #     I1b accumulates W3.h2 into the h3 half ---
                if k <= GG + 1:
                    r = rp.tile([128, 2, T], F16)
                    rt[k] = r
                    for c in range(2):
                        p1 = p1p.tile([128, T], F32, name="p1")
                        nc.tensor.matmul(
                            out=p1[:, :], lhsT=w[0:48, 0:128],
                            rhs=xsl(k)[0:48, c], start=True, stop=(k < 2),
                            tile_position=(0, 0))
                        if k >= 2:
                            nc.tensor.matmul(
                                out=p1[:, :], lhsT=w[0:64, 128:256],
                                rhs=dt[k - 2][0:64, c], start=False,
                                stop=True, tile_position=(0, 0))
                        evac(p1[:, :], r[:, c])

                # --- I2(b=k-1) + merged E2: D(b) = [h4(b-2); h2(b)] ---
                b = k - 1
                if 0 <= b <= GG + 1:
                    dd = up.tile([128, 2, T], F16)
                    dt[b] = dd
                    for c in range(2):
                        p2 = p2p.tile([128, T], F32, name="p2")
                        nc.tensor.matmul(
                            out=p2[:, :], lhsT=w[0:128, 256:384],
                            rhs=rt[b][:, c], start=True, stop=True,
                            tile_position=(0, 0))
                        evac(p2[:, :], dd[:, c])

                # --- sigma(s=k-2) ---
                s_ = k - 2
                if 0 <= s_ <= GG - 1:
                    if s_ % 16 == 0:
                        pc_tiles[s_ // 16] = pcp.tile([128, 512], F32,
                                                      name="pcb")
                    xz = dt[s_]
                    pc = pc_tiles[s_ // 16]
                    cb = (s_ % 16) * 32
                    for i in range(8):
                        c, j = i // 4, i % 4
                        nc.tensor.matmul(
                            out=pc[:, cb + i: cb + i + 1],
                            lhsT=xz[0:64, c, j * 128:(j + 1) * 128],
                            rhs=w[0:64, 448:449],
                            start=True, stop=True, tile_position=(0, 0))

                # --- I3(e=k-4): h5 psums pair-packed at partition halves ---
                e = k - 4
                if 0 <= e <= GG - 1:
                    if e % 2 == 0:
                        p3_hold[0] = [p3p.tile([128, T], F32, name="p3")
                                      for _ in range(2)]
                        lo, tp = 0, (0, 0)
                    else:
                        lo, tp = 64, (0, 64)
                    for c in range(2):
                        nc.tensor.matmul(
                            out=p3_hold[0][c][lo:lo + 64, :],
                            lhsT=w[64:128, 384:448],
                            rhs=dt[k - 2][64:128, c], start=True, stop=True,
                            tile_position=(64, tp[1]))
                    if e % 2 == 1:
                        h5p = hp.tile([128, 2, T], F16)
                        for c in range(2):
                            evac(p3_hold[0][c][:, :], h5p[:, c])
                        h5_hold[0] = (e, h5p)

                # --- color for the pair finished last iteration ---
                if h5_hold[0] is not None and h5_hold[0][0] == k - 5:
                    eo, h5p = h5_hold[0]
                    for w2, lo2 in ((eo - 1, 0), (eo, 64)):
                        pc = pc_tiles[w2 // 16]
                        cb = (w2 % 16) * 32 + 8
                        for i in range(8):
                            c, j = i // 4, i % 4
                            nc.tensor.matmul(
                                out=pc[:, cb + 3 * i: cb + 3 * i + 3],
                                lhsT=h5p[lo2:lo2 + 64, c,
                                         j * 128:(j + 1) * 128],
                                rhs=w[lo2:lo2 + 64, 449:452],
                                start=True, stop=True,
                                tile_position=(lo2, 0))

                # --- PC bank drain ---
                if k >= 20 and (k - 20) % 16 == 0:
                    g = (k - 20) // 16
                    pcs = scp.tile([128, 512], F32, name="pcs")
                    nc.vector.tensor_copy(pcs[:, :], pc_tiles[g][:, :])
                    nc.sync.dma_start(out=pcd[g], in_=pcs[:, :])

                rt.pop(k - 2, None)
                dt.pop(k - 3, None)
                xtiles.pop(k // 2 - 3, None)
    nc.finalize()
    return nc


def _get_program():
    if "nc" not in _PROG:
        _PROG["nc"] = _build_program()
    return _PROG["nc"]


def _build_weights(s0, s1, s2, c0, c1, c2, c3):
    w = np.zeros((128, 512), np.float32)
    W3 = (s2[:, 1:].astype(np.float64) @ c0[16:].astype(np.float64)
          ).astype(np.float32)
    # L1a (cols 0:128), K-rows = [pts(0:32); views(32:48)]
    w[0:32, 0:64] = s0
    w[32:48, 64:128] = c0[:16]
    # L1b (cols 128:256), K-rows 0:64 = h2 -> W3 into h3 (out cols 64:128)
    w[0:64, 128 + 64:256] = W3
    # L2 (cols 256:384): h1 -> h2 (out 0:64), h3 -> h4 (out 64:128)
    w[0:64, 256:320] = s1
    w[64:128, 320:384] = c1
    # L3/W5 (cols 384:448), K-rows 64:128 = h4 -> h5
    w[64:128, 384:448] = c2
    # sigma vector (rows 0:64 = h2), color c3 at both halves
    w[0:64, 448] = s2[:, 0]
    w[0:64, 449:452] = c3
    w[64:128, 449:452] = c3
    return w.astype(np.float16)


def _pack_input(xc):
    """xc [N_CORE, 48] f32 -> xin [M_MEGA+1, 48, 2, 2, T] f16.
    xin[m, 0:32, s, c, t]  = pts of superslot u=2m+s
    xin[m, 32:48, s, c, t] = views of superslot u-2 (zeros for u<2)."""
    xr = xc.reshape(G, 2, T, 48)
    xin = np.zeros((M_MEGA + 1, 64, 2, 2, T), np.float16)
    pts = xr[:, :, :, 0:32].transpose(0, 3, 1, 2).astype(np.float16)  # [G,32,2,T]
    vws = xr[:, :, :, 32:48].transpose(0, 3, 1, 2).astype(np.float16)
    pts_pad = np.concatenate([pts, np.zeros((2, 32, 2, T), np.float16)])
    vws_pad = np.concatenate([np.zeros((2, 16, 2, T), np.float16), vws])
    xin[:, 0:32] = pts_pad.reshape(M_MEGA + 1, 2, 32, 2, T).transpose(0, 2, 1, 3, 4)
    xin[:, 32:48] = vws_pad[:2 * (M_MEGA + 1)].reshape(
        M_MEGA + 1, 2, 16, 2, T).transpose(0, 2, 1, 3, 4)
    return xin


def _unpack_output(pcd):
    """pcd [NGEN, 128, 512] f32 -> out [N_CORE, 4] (color0..2, sigma)."""
    q = pcd.reshape(NGEN, 128, 16, 32)
    sig = q[:, :, :, 0:8].transpose(0, 2, 3, 1).reshape(N_CORE)
    col = q[:, :, :, 8:32].reshape(NGEN, 128, 16, 8, 3)
    col = col.transpose(0, 2, 3, 1, 4).reshape(N_CORE, 3)
    return np.concatenate([col, sig[:, None]], axis=1)


def kernel(x, s0, s1, s2, c0, c1, c2, c3):
    x = np.asarray(x, dtype=np.float32)
    assert x.shape == (N_PTS, 48), x.shape
    args = [np.asarray(a, dtype=np.float32) for a in (s0, s1, s2, c0, c1, c2, c3)]
    w_host = _build_weights(*args)

    in_maps = []
    for i in range(N_CORES):
        xc = x[i * N_CORE: (i + 1) * N_CORE]
        in_maps.append({"xin": _pack_input(xc), "wt": w_host})

    nc = _get_program()
    res = run_bass_kernel_spmd(nc, in_maps, core_ids=list(range(N_CORES)))

    outs = []
    for i in range(N_CORES):
        outs.append(_unpack_output(res.results[i]["pcd"]))
    return np.concatenate(outs, axis=0).astype(np.float32)
